# revision 38
# baseline (speedup 1.0000x reference)
"""GraphSAGE 2-layer forward on 8 Trainium2 NeuronCores.

Strategy (sharding_hint: partition edges by destination node):
  - Nodes padded to NP=50176 = 8 * 49 * 128, sharded by destination across
    8 cores (6272 dst nodes / 49 chunks of 128 per core).
  - Layer 1: messages x[src] are pre-gathered on the host into a per-core
    slot table (one slot per edge, tiles of 128 aligned per dst chunk) and
    STREAMED to the device with large sequential DMAs -- no descriptor
    generation on the critical path.  Aggregation uses one-hot matmuls on
    the tensor engine (oh[e, d] = (iota[d] == drel[e])), mean scaling via a
    free-dim inverse-degree multiply.
  - p = h @ W2_l (64 wide, zero-padded to 128 bf16 lanes) is written per
    chunk and exchanged with TWO AllGathers (block A = local chunks [0,25),
    block B = [25,49)) so the first exchange overlaps phase A's tail and
    the second overlaps the first wave of layer-2 gathers.
  - Layer 2: p rows are gathered on device with dma_gather.  Descriptor
    generation is the bottleneck (Q7 software DGE, ~9 ns/desc on one core
    pair), so gathers run round-robin over 4 SWDGE queues (queue q -> Q7
    core pair {2q, 2q+1}), measured ~3.3x faster than one queue.  Index
    tables are padded with -1 (trailing negatives generate no descriptors).
  - All compute matmuls run in bf16 with fp32 PSUM accumulation.
"""

import sys

sys.path.insert(0, "/opt/trn_rl_repo")

import numpy as np

N = 50000
E = 800000
D_IN, D_HID, D_OUT = 128, 128, 64
N_CORES = 8
CHUNK = 128
C_PER_CORE = 49
NODES_PC = C_PER_CORE * CHUNK  # 6272
NP_ = N_CORES * NODES_PC  # 50176
NCH = N_CORES * C_PER_CORE  # 392 chunks
NQ = 4  # SWDGE queues for layer-2 gathers
BLOCKS = [14, 14, 21]  # local chunks per p-exchange block
GROUP1 = 2  # chunks per layer-1 stream group


def _block_bounds():
    b0 = np.concatenate([[0], np.cumsum(BLOCKS)])
    return [(int(b0[i]) * CHUNK, int(b0[i + 1]) * CHUNK) for i in range(len(BLOCKS))]


def _preprocess(x, edge_index):
    """Host-side preprocessing: layer-1 message staging + layer-2 index
    tables.  Returns per-core input maps and the tile-count profiles."""
    import ml_dtypes

    bf = ml_dtypes.bfloat16
    x = np.asarray(x, dtype=np.float32)
    src = np.asarray(edge_index[0], dtype=np.int64)
    dst = np.asarray(edge_index[1], dtype=np.int64)

    cnt = np.bincount(dst, minlength=NP_).astype(np.float32)
    inv = (1.0 / np.maximum(cnt, 1.0)).astype(np.float32)

    x_pad = np.zeros((NP_, D_IN), np.float32)
    x_pad[:N] = x
    x_bf = x_pad.astype(bf)

    gchunk = dst // CHUNK  # global dst chunk of each edge
    order = np.argsort(gchunk, kind="stable")
    s_src = src[order]
    s_dst = dst[order]
    s_chunk = gchunk[order]

    n1 = np.bincount(s_chunk, minlength=NCH)  # edges per global chunk
    start1 = np.zeros(NCH, np.int64)
    start1[1:] = np.cumsum(n1)[:-1]
    T1 = (-(-n1 // 128)).reshape(N_CORES, C_PER_CORE).max(axis=0)  # per-slot
    T1_total = int(T1.sum())
    S1 = T1_total * 128
    B1 = np.zeros(C_PER_CORE, np.int64)
    B1[1:] = np.cumsum(T1)[:-1]

    # ---- layer-2: split edges per chunk by src block membership ----
    bounds = _block_bounds()
    NB = len(bounds)
    src_core = s_src // NODES_PC
    src_off = s_src % NODES_PC
    blk_of = np.zeros(len(s_src), np.int64)
    row_in_blk = np.zeros(len(s_src), np.int64)
    for b, (r0, r1) in enumerate(bounds):
        m = (src_off >= r0) & (src_off < r1)
        blk_of[m] = b
        row_in_blk[m] = src_core[m] * (r1 - r0) + (src_off[m] - r0)
    order2 = np.lexsort((blk_of, s_chunk))
    t_src_row = row_in_blk[order2]
    t_dst = s_dst[order2]
    t_blk = blk_of[order2]
    t_chunk = s_chunk[order2]

    n2 = np.zeros((NB, NCH), np.int64)
    for b in range(NB):
        n2[b] = np.bincount(t_chunk[t_blk == b], minlength=NCH)
    start2 = np.zeros(NCH, np.int64)
    start2[1:] = np.cumsum(n2.sum(axis=0))[:-1]
    TB = [
        (-(-n2[b] // 128)).reshape(N_CORES, C_PER_CORE).max(axis=0)
        for b in range(NB)
    ]

    per_core = []
    for k in range(N_CORES):
        # ---- layer-1 staged messages + drel ----
        slot_src = np.full(S1, -1, np.int64)
        drel1 = np.full(S1, 200.0, np.float32)
        for j in range(C_PER_CORE):
            c = k * C_PER_CORE + j
            nn = int(n1[c])
            s0 = int(start1[c])
            base = int(B1[j]) * 128
            slot_src[base : base + nn] = s_src[s0 : s0 + nn]
            drel1[base : base + nn] = s_dst[s0 : s0 + nn] % CHUNK
        gathered = np.zeros((S1, D_IN), bf)
        valid = slot_src >= 0
        gathered[valid] = x_bf[slot_src[valid]]
        msgs1 = np.ascontiguousarray(
            gathered.reshape(T1_total, 128, D_IN)
            .transpose(1, 0, 2)
            .reshape(128, T1_total * D_IN)
        )
        drel1_t = np.ascontiguousarray(
            drel1.reshape(T1_total, 128).T.astype(bf)
        )

        # ---- layer-2 index tables per block ----
        # trailing -1 pads generate no DMA descriptors; `cnts` carries each
        # gather's exact valid-index count (read into a register at runtime).
        idx_b = []
        drel_b = []
        cnts = np.zeros((1, NB * C_PER_CORE), np.int32)
        for b in range(NB):
            Tb = TB[b]
            Sb = int(Tb.sum()) * 128
            idx16 = np.full(Sb, -1, np.int16)
            drel = np.full(Sb, 200.0, np.float32)
            pos = 0
            for j in range(C_PER_CORE):
                c = k * C_PER_CORE + j
                nb_ = int(n2[b, c])
                s0 = int(start2[c]) + int(n2[:b, c].sum())
                idx16[pos : pos + nb_] = t_src_row[s0 : s0 + nb_]
                drel[pos : pos + nb_] = t_dst[s0 : s0 + nb_] % CHUNK
                if nb_ == 0:
                    # keep one valid dummy index so the gather is never
                    # all-negative; drel=200 zeroes its contribution.
                    idx16[pos] = 0
                    nb_ = 1
                cnts[0, b * C_PER_CORE + j] = nb_
                pos += int(Tb[j]) * 128
            assert pos == Sb
            idx_b.append(
                np.ascontiguousarray(
                    np.tile(idx16.reshape(Sb // 16, 16).T, (8, 1))
                )
            )
            drel_b.append(
                np.ascontiguousarray(
                    drel.reshape(Sb // 128, 128).T.astype(bf)
                )
            )

        inv_k = inv[k * NODES_PC : (k + 1) * NODES_PC]
        inv_rep = np.ascontiguousarray(
            np.tile(inv_k[None, :], (128, 1)).astype(bf)
        )
        inv_colT = np.ascontiguousarray(
            inv_k.reshape(C_PER_CORE, 128).T.astype(np.float32)
        )
        xT_k = np.ascontiguousarray(
            x_pad[k * NODES_PC : (k + 1) * NODES_PC].T.astype(bf)
        )
        pc = {
            "msgs1": msgs1,
            "drel1": drel1_t,
            "xT": xT_k,
            "inv_rep": inv_rep,
            "inv_colT": inv_colT,
            "cnts": cnts,
        }
        for b in range(NB):
            pc[f"idx{b}"] = idx_b[b]
            pc[f"drel2_{b}"] = drel_b[b]
        per_core.append(pc)

    return per_core, [int(v) for v in T1], [[int(v) for v in Tb] for Tb in TB]


def _shared_inputs(W1_l, b1, W1_r, W2_l, b2, W2_r):
    import ml_dtypes

    bf = ml_dtypes.bfloat16
    return {
        "W1_l": np.ascontiguousarray(np.asarray(W1_l, np.float32).astype(bf)),
        "W1_r": np.ascontiguousarray(np.asarray(W1_r, np.float32).astype(bf)),
        "W2_l": np.ascontiguousarray(np.asarray(W2_l, np.float32).astype(bf)),
        "W2_r": np.ascontiguousarray(np.asarray(W2_r, np.float32).astype(bf)),
        "b1": np.ascontiguousarray(np.asarray(b1, np.float32).reshape(D_HID, 1)),
        "b2": np.ascontiguousarray(
            np.asarray(b2, np.float32).astype(bf).reshape(1, D_OUT)
        ),
    }


def _build(T1, TB):
    import concourse.bacc as bacc
    import concourse.mybir as mybir
    from concourse.tile import TileContext

    f32 = mybir.dt.float32
    bf16 = mybir.dt.bfloat16
    i16 = mybir.dt.int16

    bounds = _block_bounds()
    NB = len(bounds)
    T1_total = sum(T1)
    TB_tot = [sum(tb) for tb in TB]
    T2max = max(max(tb) for tb in TB)

    # layer-1 stream groups (GROUP1 chunks, not crossing block boundaries)
    groups = []
    cb = 0
    for nblk in BLOCKS:
        for q in range(cb, cb + nblk, GROUP1):
            groups.append(list(range(q, min(q + GROUP1, cb + nblk))))
        cb += nblk
    PT1 = max(sum(T1[j] for j in g) for g in groups)
    PTmax = max(PT1, T2max)

    nc = bacc.Bacc(
        "TRN2",
        target_bir_lowering=False,
        debug=False,
        enable_asserts=False,
        num_devices=N_CORES,
        num_swdge_queues=NQ,
    )

    msgs1_d = nc.dram_tensor(
        "msgs1", [128, T1_total * 128], bf16, kind="ExternalInput"
    ).ap()
    drel1_d = nc.dram_tensor(
        "drel1", [128, T1_total], bf16, kind="ExternalInput"
    ).ap()
    xT_d = nc.dram_tensor("xT", [128, NODES_PC], bf16, kind="ExternalInput").ap()
    invr_d = nc.dram_tensor(
        "inv_rep", [128, NODES_PC], bf16, kind="ExternalInput"
    ).ap()
    invc_d = nc.dram_tensor(
        "inv_colT", [128, C_PER_CORE], f32, kind="ExternalInput"
    ).ap()
    w1l_d = nc.dram_tensor("W1_l", [D_IN, D_HID], bf16, kind="ExternalInput").ap()
    w1r_d = nc.dram_tensor("W1_r", [D_IN, D_HID], bf16, kind="ExternalInput").ap()
    w2l_d = nc.dram_tensor("W2_l", [D_HID, D_OUT], bf16, kind="ExternalInput").ap()
    w2r_d = nc.dram_tensor("W2_r", [D_HID, D_OUT], bf16, kind="ExternalInput").ap()
    b1_d = nc.dram_tensor("b1", [D_HID, 1], f32, kind="ExternalInput").ap()
    b2_d = nc.dram_tensor("b2", [1, D_OUT], bf16, kind="ExternalInput").ap()
    idx_d = [
        nc.dram_tensor(f"idx{b}", [128, TB_tot[b] * 8], i16, kind="ExternalInput").ap()
        for b in range(NB)
    ]
    drel2_d = [
        nc.dram_tensor(f"drel2_{b}", [128, TB_tot[b]], bf16, kind="ExternalInput").ap()
        for b in range(NB)
    ]
    cnts_d = nc.dram_tensor(
        "cnts", [1, NB * C_PER_CORE], mybir.dt.int32, kind="ExternalInput"
    ).ap()
    out_d = nc.dram_tensor(
        "out", [NODES_PC, D_OUT], f32, kind="ExternalOutput"
    ).ap()
    p_bounce = [
        nc.dram_tensor(f"p_bounce{b}", [r1 - r0, 128], bf16, kind="Internal").ap()
        for b, (r0, r1) in enumerate(bounds)
    ]
    p_full = [
        nc.dram_tensor(
            f"p_full{b}",
            [N_CORES * (r1 - r0), 128],
            bf16,
            kind="Internal",
            addr_space="Shared",
        ).ap()
        for b, (r0, r1) in enumerate(bounds)
    ]

    relu = mybir.ActivationFunctionType.Relu
    is_eq = mybir.AluOpType.is_equal
    mult = mybir.AluOpType.mult
    add = mybir.AluOpType.add

    with TileContext(nc) as tc:
        with (
            tc.tile_pool(name="persist", bufs=1) as pp,
            tc.tile_pool(name="m1", bufs=3) as m1pool,
            tc.tile_pool(name="oh1", bufs=3) as oh1pool,
            tc.tile_pool(name="m2", bufs=8) as m2pool,
            tc.tile_pool(name="oh2", bufs=6) as oh2pool,
            tc.tile_pool(name="stage", bufs=4) as spool,
            tc.tile_pool(name="psA", bufs=4, space="PSUM") as psA,
            tc.tile_pool(name="psH", bufs=2, space="PSUM") as psH,
            tc.tile_pool(name="psO", bufs=2, space="PSUM") as psO,
        ):
            xT_sb = pp.tile([128, NODES_PC], bf16)
            nc.sync.dma_start(out=xT_sb[:], in_=xT_d)
            drel1_sb = pp.tile([128, T1_total], bf16)
            nc.sync.dma_start(out=drel1_sb[:], in_=drel1_d)
            idx_sb = []
            drel2_sb = []
            for b in range(NB):
                ti = pp.tile([128, TB_tot[b] * 8], i16, name=f"idx_sb{b}")
                nc.sync.dma_start(out=ti[:], in_=idx_d[b])
                idx_sb.append(ti)
                td = pp.tile([128, TB_tot[b]], bf16, name=f"drel2_sb{b}")
                nc.sync.dma_start(out=td[:], in_=drel2_d[b])
                drel2_sb.append(td)
            invr_sb = pp.tile([128, NODES_PC], bf16)
            nc.sync.dma_start(out=invr_sb[:], in_=invr_d)
            cnts_sb = pp.tile([1, NB * C_PER_CORE], mybir.dt.int32)
            nc.sync.dma_start(out=cnts_sb[:], in_=cnts_d)

            invc_sb = pp.tile([128, C_PER_CORE], f32)
            nc.sync.dma_start(out=invc_sb[:], in_=invc_d)
            w1l_sb = pp.tile([D_IN, D_HID], bf16)
            nc.sync.dma_start(out=w1l_sb[:], in_=w1l_d)
            w1r_sb = pp.tile([D_IN, D_HID], bf16)
            nc.sync.dma_start(out=w1r_sb[:], in_=w1r_d)
            w2l_sb = pp.tile([D_HID, D_OUT], bf16)
            nc.sync.dma_start(out=w2l_sb[:], in_=w2l_d)
            w2r_sb = pp.tile([D_HID, D_OUT], bf16)
            nc.sync.dma_start(out=w2r_sb[:], in_=w2r_d)
            b1_sb = pp.tile([D_HID, 1], f32)
            nc.sync.dma_start(out=b1_sb[:], in_=b1_d)
            b2_sb = pp.tile([1, D_OUT], bf16)
            nc.sync.dma_start(out=b2_sb[:], in_=b2_d)
            iota_sb = pp.tile([128, 128], f32)
            nc.gpsimd.iota(
                iota_sb[:],
                pattern=[[1, 128]],
                base=0,
                channel_multiplier=0,
                allow_small_or_imprecise_dtypes=True,
            )
            iota16 = pp.tile([128, 128], bf16)
            nc.vector.tensor_copy(out=iota16[:], in_=iota_sb[:])
            iota_rep = pp.tile([128, PTmax * 128], bf16)
            for t in range(PTmax):
                nc.scalar.copy(
                    out=iota_rep[:, t * 128 : (t + 1) * 128], in_=iota16[:]
                )
            ones_sb = pp.tile([1, 128], bf16)
            nc.vector.memset(ones_sb[:], 1.0)
            h_all = pp.tile([128, NODES_PC], bf16)
            aggp = pp.tile([128, C_PER_CORE * D_OUT], f32)

            # pre-zero the gather landing buffers: trailing -1 slots are
            # never written by the DMA, and the one-hot zeros them in the
            # matmul, so they only need to be finite.
            for i in range(8):
                mz = m2pool.tile(
                    [128, T2max * 128], bf16, tag="m2", name=f"m2_init{i}"
                )
                nc.gpsimd.memset(mz[:], 0.0)



            # ---------------- phase A: layer 1 + p = h @ W2_l ----------------
            def chunk_block(j):
                for b, nblk in enumerate(BLOCKS):
                    j -= nblk
                    if j < 0:
                        return b, j + nblk
                raise AssertionError

            # group tile-offsets, msg loads and one-hot builds are emitted one
            # group AHEAD of their consumers so the DVE stream never stalls
            # behind a consumer op that waits on the tensor engine (convoy).
            gbase = []
            acc = 0
            for g in groups:
                gbase.append(acc)
                acc += sum(T1[j] for j in g)

            def emit_group_load(gi):
                g = groups[gi]
                ttp = sum(T1[j] for j in g)
                tb = gbase[gi]
                msg1 = m1pool.tile(
                    [128, PT1 * 128], bf16, tag="m1", name=f"m1_{gi}"
                )
                nc.sync.dma_start(
                    out=msg1[:, : ttp * 128],
                    in_=msgs1_d[:, tb * 128 : (tb + ttp) * 128],
                )
                oh = oh1pool.tile(
                    [128, PT1 * 128], bf16, tag="oh1", name=f"oh1_{gi}"
                )
                nc.vector.tensor_tensor(
                    out=oh[:, : ttp * 128].rearrange("p (t e) -> p t e", e=128),
                    in0=iota_rep[:, : ttp * 128].rearrange(
                        "p (t e) -> p t e", e=128
                    ),
                    in1=drel1_sb[:, tb : tb + ttp]
                    .rearrange("p (t e) -> p t e", e=1)
                    .broadcast_to([128, ttp, 128]),
                    op=is_eq,
                )
                return msg1, oh

            done_blk = 0
            pend = emit_group_load(0)
            for gi, g in enumerate(groups):
                msg1, oh = pend
                if gi + 1 < len(groups):
                    pend = emit_group_load(gi + 1)
                off = 0
                for j in g:
                    jsl = slice(j * 128, (j + 1) * 128)
                    pa = psA.tile([128, 128], f32, tag="agg")
                    for ci in range(T1[j]):
                        t = off + ci
                        nc.tensor.matmul(
                            out=pa[:],
                            lhsT=msg1[:, t * 128 : (t + 1) * 128],
                            rhs=oh[:, t * 128 : (t + 1) * 128],
                            start=(ci == 0),
                            stop=(ci == T1[j] - 1),
                        )
                    meanT = spool.tile([128, 128], bf16, tag="meanT")
                    nc.vector.tensor_tensor(
                        out=meanT[:], in0=pa[:], in1=invr_sb[:, jsl], op=mult
                    )
                    ph = psH.tile([128, 128], f32, tag="h")
                    nc.tensor.matmul(
                        out=ph[:], lhsT=w1l_sb[:], rhs=meanT[:],
                        start=True, stop=False,
                    )
                    nc.tensor.matmul(
                        out=ph[:], lhsT=w1r_sb[:], rhs=xT_sb[:, jsl],
                        start=False, stop=True,
                    )
                    nc.scalar.activation(
                        out=h_all[:, jsl], in_=ph[:], func=relu,
                        bias=b1_sb[:, 0:1], scale=1.0,
                    )
                    po = psO.tile([128, D_OUT], f32, tag="p")
                    nc.tensor.matmul(
                        out=po[:], lhsT=h_all[:, jsl], rhs=w2l_sb[:],
                        start=True, stop=True,
                    )
                    # pad columns zeroed on the (phase-A-idle) gpsimd engine
                    p_sb = spool.tile([128, 128], bf16, tag="p_sb")
                    nc.gpsimd.memset(p_sb[:, D_OUT:128], 0.0)
                    nc.scalar.copy(out=p_sb[:, 0:D_OUT], in_=po[:])
                    b, jloc = chunk_block(j)
                    nc.sync.dma_start(
                        out=p_bounce[b][jloc * 128 : (jloc + 1) * 128, :],
                        in_=p_sb[:],
                    )
                    off += T1[j]
                # fire the block's AllGather as soon as its last chunk is out
                if g[-1] == sum(BLOCKS[: done_blk + 1]) - 1:
                    nc.gpsimd.collective_compute(
                        "AllGather",
                        mybir.AluOpType.bypass,
                        replica_groups=[list(range(N_CORES))],
                        ins=[p_bounce[done_blk][:]],
                        outs=[p_full[done_blk]],
                    )
                    done_blk += 1

            # ---------------- phase B: layer 2, one wave per block ----------------
            # chunk sequence across both waves, with per-(b, j) tile offsets.
            seq = [(b, j) for b in range(NB) for j in range(C_PER_CORE)]
            tbb_of = {}
            for b in range(NB):
                acc2 = 0
                for j in range(C_PER_CORE):
                    tbb_of[(b, j)] = acc2
                    acc2 += TB[b][j]

            KLOOK = 5  # one-hot builds run this many chunks ahead of their use

            def emit_oh2(si):
                b, j = seq[si]
                Tb = TB[b][j]
                tbb = tbb_of[(b, j)]
                oh = oh2pool.tile(
                    [128, T2max * 128], bf16, tag="oh2", name=f"oh2_{b}_{j}"
                )
                nc.vector.tensor_tensor(
                    out=oh[:, : Tb * 128].rearrange("p (t e) -> p t e", e=128),
                    in0=iota_rep[:, : Tb * 128].rearrange(
                        "p (t e) -> p t e", e=128
                    ),
                    in1=drel2_sb[b][:, tbb : tbb + Tb]
                    .rearrange("p (t e) -> p t e", e=1)
                    .broadcast_to([128, Tb, 128]),
                    op=is_eq,
                )
                return oh

            oh_ahead = [emit_oh2(si) for si in range(KLOOK)]
            cnt_regs = [
                nc.gpsimd.alloc_register(f"cnt_reg{i}") for i in range(2)
            ]
            for si, (b, j) in enumerate(seq):
                Tb = TB[b][j]
                tbb = tbb_of[(b, j)]
                rows_b = N_CORES * (bounds[b][1] - bounds[b][0])
                jsl = slice(j * 128, (j + 1) * 128)
                osl = slice(j * D_OUT, (j + 1) * D_OUT)
                msg2 = m2pool.tile([128, T2max * 128], bf16, tag="m2")
                # exact per-core valid-index count: the SWDGE ring bookkeeping
                # requires num_idxs_reg == number of descriptors generated
                # (a conservative padded count with -1 pads hangs the device)
                cnt_reg = cnt_regs[si % 2]
                nc.gpsimd.reg_load(cnt_reg, cnts_sb[0:1, si : si + 1])
                nc.gpsimd.dma_gather(
                    out_ap=msg2[:, : Tb * 128].rearrange(
                        "p (t e) -> p t e", e=128
                    ),
                    in_ap=p_full[b][0:rows_b, :],
                    idxs_ap=idx_sb[b][:, tbb * 8 : (tbb + Tb) * 8],
                    num_idxs=Tb * 128,
                    num_idxs_reg=cnt_reg,
                    elem_size=128,
                    single_packet=False,
                    queue_num=si % NQ,
                )
                oh = oh_ahead[si % KLOOK]
                if si + KLOOK < len(seq):
                    oh_ahead[si % KLOOK] = emit_oh2(si + KLOOK)
                pf = psA.tile([128, 128], f32, tag="agg")
                for ci in range(Tb):
                    nc.tensor.matmul(
                        out=pf[:],
                        lhsT=oh[:, ci * 128 : (ci + 1) * 128],
                        rhs=msg2[:, ci * 128 : (ci + 1) * 128],
                        start=(ci == 0),
                        stop=(ci == Tb - 1),
                    )
                if True:
                    if b == 0:
                        pd = psO.tile([128, D_OUT], f32, tag="p")
                        nc.tensor.matmul(
                            out=pd[:], lhsT=h_all[:, jsl], rhs=w2r_sb[:],
                            start=True, stop=False,
                        )
                        nc.tensor.matmul(
                            out=pd[:], lhsT=ones_sb[:], rhs=b2_sb[:],
                            start=False, stop=True,
                        )
                        pd_sb = spool.tile([128, D_OUT], f32, tag="pd_sb")
                        nc.scalar.copy(out=pd_sb[:], in_=pd[:])
                        nc.vector.scalar_tensor_tensor(
                            out=aggp[:, osl],
                            in0=pf[:, 0:D_OUT],
                            scalar=invc_sb[:, j : j + 1],
                            in1=pd_sb[:],
                            op0=mult,
                            op1=add,
                        )
                    elif b < NB - 1:
                        nc.vector.scalar_tensor_tensor(
                            out=aggp[:, osl],
                            in0=pf[:, 0:D_OUT],
                            scalar=invc_sb[:, j : j + 1],
                            in1=aggp[:, osl],
                            op0=mult,
                            op1=add,
                        )
                    else:
                        out_sb = spool.tile([128, D_OUT], f32, tag="out_sb")
                        nc.vector.scalar_tensor_tensor(
                            out=out_sb[:],
                            in0=pf[:, 0:D_OUT],
                            scalar=invc_sb[:, j : j + 1],
                            in1=aggp[:, osl],
                            op0=mult,
                            op1=add,
                        )
                        nc.sync.dma_start(out=out_d[jsl, :], in_=out_sb[:])

    nc.compile()
    return nc


def _prepare(x, edge_index, W1_l, b1, W1_r, W2_l, b2, W2_r):
    per_core, T1, TB = _preprocess(x, edge_index)
    nc = _build(T1, TB)
    shared = _shared_inputs(W1_l, b1, W1_r, W2_l, b2, W2_r)
    in_maps = [{**pc, **shared} for pc in per_core]
    return nc, in_maps


def kernel(
    x,
    edge_index,
    W1_l,
    b1,
    W1_r,
    W2_l,
    b2,
    W2_r,
):
    from concourse.bass_utils import run_bass_kernel_spmd

    nc, in_maps = _prepare(x, edge_index, W1_l, b1, W1_r, W2_l, b2, W2_r)
    res = run_bass_kernel_spmd(nc, in_maps, core_ids=list(range(N_CORES)))
    out = np.concatenate([r["out"] for r in res.results], axis=0)
    return out[:N].astype(np.float32)


if __name__ == "__main__":
    rng = np.random.default_rng(0)
    x = rng.standard_normal((N, D_IN), dtype=np.float32)
    ei = rng.integers(0, N, size=(2, E), dtype=np.int64)
    s = 1.0 / np.sqrt(D_IN)
    w1l = rng.uniform(-s, s, (D_IN, D_HID)).astype(np.float32)
    w1r = rng.uniform(-s, s, (D_IN, D_HID)).astype(np.float32)
    s2 = 1.0 / np.sqrt(D_HID)
    w2l = rng.uniform(-s2, s2, (D_HID, D_OUT)).astype(np.float32)
    w2r = rng.uniform(-s2, s2, (D_HID, D_OUT)).astype(np.float32)
    out = kernel(
        x=x,
        edge_index=ei,
        W1_l=w1l,
        b1=np.zeros(D_HID, np.float32),
        W1_r=w1r,
        W2_l=w2l,
        b2=np.zeros(D_OUT, np.float32),
        W2_r=w2r,
    )
    print(out.shape, out.dtype)


# revision 39
# speedup vs baseline: 1.0364x; 1.0364x over previous
"""GraphSAGE 2-layer forward on 8 Trainium2 NeuronCores.

Strategy (sharding_hint: partition edges by destination node):
  - Nodes padded to NP=50176 = 8 * 49 * 128, sharded by destination across
    8 cores (6272 dst nodes / 49 chunks of 128 per core).
  - Layer 1: messages x[src] are pre-gathered on the host into a per-core
    slot table (one slot per edge, tiles of 128 aligned per dst chunk) and
    STREAMED to the device with large sequential DMAs -- no descriptor
    generation on the critical path.  Aggregation uses one-hot matmuls on
    the tensor engine (oh[e, d] = (iota[d] == drel[e])), mean scaling via a
    free-dim inverse-degree multiply.
  - p = h @ W2_l (64 wide, zero-padded to 128 bf16 lanes) is written per
    chunk and exchanged with TWO AllGathers (block A = local chunks [0,25),
    block B = [25,49)) so the first exchange overlaps phase A's tail and
    the second overlaps the first wave of layer-2 gathers.
  - Layer 2: p rows are gathered on device with dma_gather.  Descriptor
    generation is the bottleneck (Q7 software DGE, ~9 ns/desc on one core
    pair), so gathers run round-robin over 4 SWDGE queues (queue q -> Q7
    core pair {2q, 2q+1}), measured ~3.3x faster than one queue.  Index
    tables are padded with -1 (trailing negatives generate no descriptors).
  - All compute matmuls run in bf16 with fp32 PSUM accumulation.
"""

import sys

sys.path.insert(0, "/opt/trn_rl_repo")

import numpy as np

N = 50000
E = 800000
D_IN, D_HID, D_OUT = 128, 128, 64
N_CORES = 8
CHUNK = 128
C_PER_CORE = 49
NODES_PC = C_PER_CORE * CHUNK  # 6272
NP_ = N_CORES * NODES_PC  # 50176
NCH = N_CORES * C_PER_CORE  # 392 chunks
NQ = 4  # SWDGE queues for layer-2 gathers
BLOCKS = [27, 22]  # local chunks per p-exchange block
GROUP1 = 2  # chunks per layer-1 stream group


def _block_bounds():
    b0 = np.concatenate([[0], np.cumsum(BLOCKS)])
    return [(int(b0[i]) * CHUNK, int(b0[i + 1]) * CHUNK) for i in range(len(BLOCKS))]


def _preprocess(x, edge_index):
    """Host-side preprocessing: layer-1 message staging + layer-2 index
    tables.  Returns per-core input maps and the tile-count profiles."""
    import ml_dtypes

    bf = ml_dtypes.bfloat16
    x = np.asarray(x, dtype=np.float32)
    src = np.asarray(edge_index[0], dtype=np.int64)
    dst = np.asarray(edge_index[1], dtype=np.int64)

    cnt = np.bincount(dst, minlength=NP_).astype(np.float32)
    inv = (1.0 / np.maximum(cnt, 1.0)).astype(np.float32)

    x_pad = np.zeros((NP_, D_IN), np.float32)
    x_pad[:N] = x
    x_bf = x_pad.astype(bf)

    gchunk = dst // CHUNK  # global dst chunk of each edge
    order = np.argsort(gchunk, kind="stable")
    s_src = src[order]
    s_dst = dst[order]
    s_chunk = gchunk[order]

    n1 = np.bincount(s_chunk, minlength=NCH)  # edges per global chunk
    start1 = np.zeros(NCH, np.int64)
    start1[1:] = np.cumsum(n1)[:-1]
    T1 = (-(-n1 // 128)).reshape(N_CORES, C_PER_CORE).max(axis=0)  # per-slot
    T1_total = int(T1.sum())
    S1 = T1_total * 128
    B1 = np.zeros(C_PER_CORE, np.int64)
    B1[1:] = np.cumsum(T1)[:-1]

    # ---- layer-2: split edges per chunk by src block membership ----
    bounds = _block_bounds()
    NB = len(bounds)
    src_core = s_src // NODES_PC
    src_off = s_src % NODES_PC
    blk_of = np.zeros(len(s_src), np.int64)
    row_in_blk = np.zeros(len(s_src), np.int64)
    for b, (r0, r1) in enumerate(bounds):
        m = (src_off >= r0) & (src_off < r1)
        blk_of[m] = b
        row_in_blk[m] = src_core[m] * (r1 - r0) + (src_off[m] - r0)
    order2 = np.lexsort((blk_of, s_chunk))
    t_src_row = row_in_blk[order2]
    t_dst = s_dst[order2]
    t_blk = blk_of[order2]
    t_chunk = s_chunk[order2]

    n2 = np.zeros((NB, NCH), np.int64)
    for b in range(NB):
        n2[b] = np.bincount(t_chunk[t_blk == b], minlength=NCH)
    start2 = np.zeros(NCH, np.int64)
    start2[1:] = np.cumsum(n2.sum(axis=0))[:-1]
    TB = [
        (-(-n2[b] // 128)).reshape(N_CORES, C_PER_CORE).max(axis=0)
        for b in range(NB)
    ]

    per_core = []
    for k in range(N_CORES):
        # ---- layer-1 staged messages + drel ----
        slot_src = np.full(S1, -1, np.int64)
        drel1 = np.full(S1, 200.0, np.float32)
        for j in range(C_PER_CORE):
            c = k * C_PER_CORE + j
            nn = int(n1[c])
            s0 = int(start1[c])
            base = int(B1[j]) * 128
            slot_src[base : base + nn] = s_src[s0 : s0 + nn]
            drel1[base : base + nn] = s_dst[s0 : s0 + nn] % CHUNK
        gathered = np.zeros((S1, D_IN), bf)
        valid = slot_src >= 0
        gathered[valid] = x_bf[slot_src[valid]]
        msgs1 = np.ascontiguousarray(
            gathered.reshape(T1_total, 128, D_IN)
            .transpose(1, 0, 2)
            .reshape(128, T1_total * D_IN)
        )
        drel1_t = np.ascontiguousarray(
            drel1.reshape(T1_total, 128).T.astype(bf)
        )

        # ---- layer-2 index tables per block ----
        # trailing -1 pads generate no DMA descriptors; `cnts` carries each
        # gather's exact valid-index count (read into a register at runtime).
        idx_b = []
        drel_b = []
        cnts = np.zeros((1, NB * C_PER_CORE), np.int32)
        for b in range(NB):
            Tb = TB[b]
            Sb = int(Tb.sum()) * 128
            idx16 = np.full(Sb, -1, np.int16)
            drel = np.full(Sb, 200.0, np.float32)
            pos = 0
            for j in range(C_PER_CORE):
                c = k * C_PER_CORE + j
                nb_ = int(n2[b, c])
                s0 = int(start2[c]) + int(n2[:b, c].sum())
                idx16[pos : pos + nb_] = t_src_row[s0 : s0 + nb_]
                drel[pos : pos + nb_] = t_dst[s0 : s0 + nb_] % CHUNK
                if nb_ == 0:
                    # keep one valid dummy index so the gather is never
                    # all-negative; drel=200 zeroes its contribution.
                    idx16[pos] = 0
                    nb_ = 1
                cnts[0, b * C_PER_CORE + j] = nb_
                pos += int(Tb[j]) * 128
            assert pos == Sb
            idx_b.append(
                np.ascontiguousarray(
                    np.tile(idx16.reshape(Sb // 16, 16).T, (8, 1))
                )
            )
            drel_b.append(
                np.ascontiguousarray(
                    drel.reshape(Sb // 128, 128).T.astype(bf)
                )
            )

        inv_k = inv[k * NODES_PC : (k + 1) * NODES_PC]
        inv_rep = np.ascontiguousarray(
            np.tile(inv_k[None, :], (128, 1)).astype(bf)
        )
        inv_colT = np.ascontiguousarray(
            inv_k.reshape(C_PER_CORE, 128).T.astype(np.float32)
        )
        xT_k = np.ascontiguousarray(
            x_pad[k * NODES_PC : (k + 1) * NODES_PC].T.astype(bf)
        )
        pc = {
            "msgs1": msgs1,
            "drel1": drel1_t,
            "xT": xT_k,
            "inv_rep": inv_rep,
            "inv_colT": inv_colT,
            "cnts": cnts,
        }
        for b in range(NB):
            pc[f"idx{b}"] = idx_b[b]
            pc[f"drel2_{b}"] = drel_b[b]
        per_core.append(pc)

    return per_core, [int(v) for v in T1], [[int(v) for v in Tb] for Tb in TB]


def _shared_inputs(W1_l, b1, W1_r, W2_l, b2, W2_r):
    import ml_dtypes

    bf = ml_dtypes.bfloat16
    return {
        "W1_l": np.ascontiguousarray(np.asarray(W1_l, np.float32).astype(bf)),
        "W1_r": np.ascontiguousarray(np.asarray(W1_r, np.float32).astype(bf)),
        "W2_l": np.ascontiguousarray(np.asarray(W2_l, np.float32).astype(bf)),
        "W2_r": np.ascontiguousarray(np.asarray(W2_r, np.float32).astype(bf)),
        "b1": np.ascontiguousarray(np.asarray(b1, np.float32).reshape(D_HID, 1)),
        "b2": np.ascontiguousarray(
            np.asarray(b2, np.float32).astype(bf).reshape(1, D_OUT)
        ),
    }


def _build(T1, TB):
    import concourse.bacc as bacc
    import concourse.mybir as mybir
    from concourse.tile import TileContext

    f32 = mybir.dt.float32
    bf16 = mybir.dt.bfloat16
    i16 = mybir.dt.int16

    bounds = _block_bounds()
    NB = len(bounds)
    T1_total = sum(T1)
    TB_tot = [sum(tb) for tb in TB]
    T2max = max(max(tb) for tb in TB)

    # layer-1 stream groups (GROUP1 chunks, not crossing block boundaries)
    groups = []
    cb = 0
    for nblk in BLOCKS:
        for q in range(cb, cb + nblk, GROUP1):
            groups.append(list(range(q, min(q + GROUP1, cb + nblk))))
        cb += nblk
    PT1 = max(sum(T1[j] for j in g) for g in groups)
    PTmax = max(PT1, T2max)

    nc = bacc.Bacc(
        "TRN2",
        target_bir_lowering=False,
        debug=False,
        enable_asserts=False,
        num_devices=N_CORES,
        num_swdge_queues=NQ,
    )

    msgs1_d = nc.dram_tensor(
        "msgs1", [128, T1_total * 128], bf16, kind="ExternalInput"
    ).ap()
    drel1_d = nc.dram_tensor(
        "drel1", [128, T1_total], bf16, kind="ExternalInput"
    ).ap()
    xT_d = nc.dram_tensor("xT", [128, NODES_PC], bf16, kind="ExternalInput").ap()
    invr_d = nc.dram_tensor(
        "inv_rep", [128, NODES_PC], bf16, kind="ExternalInput"
    ).ap()
    invc_d = nc.dram_tensor(
        "inv_colT", [128, C_PER_CORE], f32, kind="ExternalInput"
    ).ap()
    w1l_d = nc.dram_tensor("W1_l", [D_IN, D_HID], bf16, kind="ExternalInput").ap()
    w1r_d = nc.dram_tensor("W1_r", [D_IN, D_HID], bf16, kind="ExternalInput").ap()
    w2l_d = nc.dram_tensor("W2_l", [D_HID, D_OUT], bf16, kind="ExternalInput").ap()
    w2r_d = nc.dram_tensor("W2_r", [D_HID, D_OUT], bf16, kind="ExternalInput").ap()
    b1_d = nc.dram_tensor("b1", [D_HID, 1], f32, kind="ExternalInput").ap()
    b2_d = nc.dram_tensor("b2", [1, D_OUT], bf16, kind="ExternalInput").ap()
    idx_d = [
        nc.dram_tensor(f"idx{b}", [128, TB_tot[b] * 8], i16, kind="ExternalInput").ap()
        for b in range(NB)
    ]
    drel2_d = [
        nc.dram_tensor(f"drel2_{b}", [128, TB_tot[b]], bf16, kind="ExternalInput").ap()
        for b in range(NB)
    ]
    cnts_d = nc.dram_tensor(
        "cnts", [1, NB * C_PER_CORE], mybir.dt.int32, kind="ExternalInput"
    ).ap()
    out_d = nc.dram_tensor(
        "out", [NODES_PC, D_OUT], f32, kind="ExternalOutput"
    ).ap()
    p_bounce = [
        nc.dram_tensor(f"p_bounce{b}", [r1 - r0, 128], bf16, kind="Internal").ap()
        for b, (r0, r1) in enumerate(bounds)
    ]
    p_full = [
        nc.dram_tensor(
            f"p_full{b}",
            [N_CORES * (r1 - r0), 128],
            bf16,
            kind="Internal",
            addr_space="Shared",
        ).ap()
        for b, (r0, r1) in enumerate(bounds)
    ]

    relu = mybir.ActivationFunctionType.Relu
    is_eq = mybir.AluOpType.is_equal
    mult = mybir.AluOpType.mult
    add = mybir.AluOpType.add

    with TileContext(nc) as tc:
        with (
            tc.tile_pool(name="persist", bufs=1) as pp,
            tc.tile_pool(name="m1", bufs=3) as m1pool,
            tc.tile_pool(name="oh1", bufs=3) as oh1pool,
            tc.tile_pool(name="m2", bufs=8) as m2pool,
            tc.tile_pool(name="oh2", bufs=6) as oh2pool,
            tc.tile_pool(name="stage", bufs=4) as spool,
            tc.tile_pool(name="psA", bufs=4, space="PSUM") as psA,
            tc.tile_pool(name="psH", bufs=2, space="PSUM") as psH,
            tc.tile_pool(name="psO", bufs=2, space="PSUM") as psO,
        ):
            xT_sb = pp.tile([128, NODES_PC], bf16)
            nc.sync.dma_start(out=xT_sb[:], in_=xT_d)
            drel1_sb = pp.tile([128, T1_total], bf16)
            nc.sync.dma_start(out=drel1_sb[:], in_=drel1_d)
            idx_sb = []
            drel2_sb = []
            for b in range(NB):
                ti = pp.tile([128, TB_tot[b] * 8], i16, name=f"idx_sb{b}")
                nc.sync.dma_start(out=ti[:], in_=idx_d[b])
                idx_sb.append(ti)
                td = pp.tile([128, TB_tot[b]], bf16, name=f"drel2_sb{b}")
                nc.sync.dma_start(out=td[:], in_=drel2_d[b])
                drel2_sb.append(td)
            invr_sb = pp.tile([128, NODES_PC], bf16)
            nc.sync.dma_start(out=invr_sb[:], in_=invr_d)
            cnts_sb = pp.tile([1, NB * C_PER_CORE], mybir.dt.int32)
            nc.sync.dma_start(out=cnts_sb[:], in_=cnts_d)

            invc_sb = pp.tile([128, C_PER_CORE], f32)
            nc.sync.dma_start(out=invc_sb[:], in_=invc_d)
            w1l_sb = pp.tile([D_IN, D_HID], bf16)
            nc.sync.dma_start(out=w1l_sb[:], in_=w1l_d)
            w1r_sb = pp.tile([D_IN, D_HID], bf16)
            nc.sync.dma_start(out=w1r_sb[:], in_=w1r_d)
            w2l_sb = pp.tile([D_HID, D_OUT], bf16)
            nc.sync.dma_start(out=w2l_sb[:], in_=w2l_d)
            w2r_sb = pp.tile([D_HID, D_OUT], bf16)
            nc.sync.dma_start(out=w2r_sb[:], in_=w2r_d)
            b1_sb = pp.tile([D_HID, 1], f32)
            nc.sync.dma_start(out=b1_sb[:], in_=b1_d)
            b2_sb = pp.tile([1, D_OUT], bf16)
            nc.sync.dma_start(out=b2_sb[:], in_=b2_d)
            iota_sb = pp.tile([128, 128], f32)
            nc.gpsimd.iota(
                iota_sb[:],
                pattern=[[1, 128]],
                base=0,
                channel_multiplier=0,
                allow_small_or_imprecise_dtypes=True,
            )
            iota16 = pp.tile([128, 128], bf16)
            nc.vector.tensor_copy(out=iota16[:], in_=iota_sb[:])
            iota_rep = pp.tile([128, PTmax * 128], bf16)
            for t in range(PTmax):
                nc.scalar.copy(
                    out=iota_rep[:, t * 128 : (t + 1) * 128], in_=iota16[:]
                )
            ones_sb = pp.tile([1, 128], bf16)
            nc.vector.memset(ones_sb[:], 1.0)
            h_all = pp.tile([128, NODES_PC], bf16)
            aggp = pp.tile([128, C_PER_CORE * D_OUT], f32)

            # pre-zero the gather landing buffers: trailing -1 slots are
            # never written by the DMA, and the one-hot zeros them in the
            # matmul, so they only need to be finite.
            for i in range(8):
                mz = m2pool.tile(
                    [128, T2max * 128], bf16, tag="m2", name=f"m2_init{i}"
                )
                nc.gpsimd.memset(mz[:], 0.0)



            # ---------------- phase A: layer 1 + p = h @ W2_l ----------------
            def chunk_block(j):
                for b, nblk in enumerate(BLOCKS):
                    j -= nblk
                    if j < 0:
                        return b, j + nblk
                raise AssertionError

            # group tile-offsets, msg loads and one-hot builds are emitted one
            # group AHEAD of their consumers so the DVE stream never stalls
            # behind a consumer op that waits on the tensor engine (convoy).
            gbase = []
            acc = 0
            for g in groups:
                gbase.append(acc)
                acc += sum(T1[j] for j in g)

            def emit_group_load(gi):
                g = groups[gi]
                ttp = sum(T1[j] for j in g)
                tb = gbase[gi]
                msg1 = m1pool.tile(
                    [128, PT1 * 128], bf16, tag="m1", name=f"m1_{gi}"
                )
                nc.sync.dma_start(
                    out=msg1[:, : ttp * 128],
                    in_=msgs1_d[:, tb * 128 : (tb + ttp) * 128],
                )
                oh = oh1pool.tile(
                    [128, PT1 * 128], bf16, tag="oh1", name=f"oh1_{gi}"
                )
                nc.vector.tensor_tensor(
                    out=oh[:, : ttp * 128].rearrange("p (t e) -> p t e", e=128),
                    in0=iota_rep[:, : ttp * 128].rearrange(
                        "p (t e) -> p t e", e=128
                    ),
                    in1=drel1_sb[:, tb : tb + ttp]
                    .rearrange("p (t e) -> p t e", e=1)
                    .broadcast_to([128, ttp, 128]),
                    op=is_eq,
                )
                return msg1, oh

            done_blk = 0
            pend = emit_group_load(0)
            for gi, g in enumerate(groups):
                msg1, oh = pend
                if gi + 1 < len(groups):
                    pend = emit_group_load(gi + 1)
                off = 0
                for j in g:
                    jsl = slice(j * 128, (j + 1) * 128)
                    pa = psA.tile([128, 128], f32, tag="agg")
                    for ci in range(T1[j]):
                        t = off + ci
                        nc.tensor.matmul(
                            out=pa[:],
                            lhsT=msg1[:, t * 128 : (t + 1) * 128],
                            rhs=oh[:, t * 128 : (t + 1) * 128],
                            start=(ci == 0),
                            stop=(ci == T1[j] - 1),
                        )
                    meanT = spool.tile([128, 128], bf16, tag="meanT")
                    nc.vector.tensor_tensor(
                        out=meanT[:], in0=pa[:], in1=invr_sb[:, jsl], op=mult
                    )
                    ph = psH.tile([128, 128], f32, tag="h")
                    nc.tensor.matmul(
                        out=ph[:], lhsT=w1l_sb[:], rhs=meanT[:],
                        start=True, stop=False,
                    )
                    nc.tensor.matmul(
                        out=ph[:], lhsT=w1r_sb[:], rhs=xT_sb[:, jsl],
                        start=False, stop=True,
                    )
                    nc.scalar.activation(
                        out=h_all[:, jsl], in_=ph[:], func=relu,
                        bias=b1_sb[:, 0:1], scale=1.0,
                    )
                    po = psO.tile([128, D_OUT], f32, tag="p")
                    nc.tensor.matmul(
                        out=po[:], lhsT=h_all[:, jsl], rhs=w2l_sb[:],
                        start=True, stop=True,
                    )
                    # pad columns zeroed on the (phase-A-idle) gpsimd engine
                    p_sb = spool.tile([128, 128], bf16, tag="p_sb")
                    nc.gpsimd.memset(p_sb[:, D_OUT:128], 0.0)
                    nc.scalar.copy(out=p_sb[:, 0:D_OUT], in_=po[:])
                    b, jloc = chunk_block(j)
                    nc.sync.dma_start(
                        out=p_bounce[b][jloc * 128 : (jloc + 1) * 128, :],
                        in_=p_sb[:],
                    )
                    off += T1[j]
                # fire the block's AllGather as soon as its last chunk is out
                if g[-1] == sum(BLOCKS[: done_blk + 1]) - 1:
                    nc.gpsimd.collective_compute(
                        "AllGather",
                        mybir.AluOpType.bypass,
                        replica_groups=[list(range(N_CORES))],
                        ins=[p_bounce[done_blk][:]],
                        outs=[p_full[done_blk]],
                    )
                    done_blk += 1

            # ---------------- phase B: layer 2, one wave per block ----------------
            # chunk sequence across both waves, with per-(b, j) tile offsets.
            seq = [(b, j) for b in range(NB) for j in range(C_PER_CORE)]
            tbb_of = {}
            for b in range(NB):
                acc2 = 0
                for j in range(C_PER_CORE):
                    tbb_of[(b, j)] = acc2
                    acc2 += TB[b][j]

            KLOOK = 5  # one-hot builds run this many chunks ahead of their use

            def emit_oh2(si):
                b, j = seq[si]
                Tb = TB[b][j]
                tbb = tbb_of[(b, j)]
                oh = oh2pool.tile(
                    [128, T2max * 128], bf16, tag="oh2", name=f"oh2_{b}_{j}"
                )
                nc.vector.tensor_tensor(
                    out=oh[:, : Tb * 128].rearrange("p (t e) -> p t e", e=128),
                    in0=iota_rep[:, : Tb * 128].rearrange(
                        "p (t e) -> p t e", e=128
                    ),
                    in1=drel2_sb[b][:, tbb : tbb + Tb]
                    .rearrange("p (t e) -> p t e", e=1)
                    .broadcast_to([128, Tb, 128]),
                    op=is_eq,
                )
                return oh

            oh_ahead = [emit_oh2(si) for si in range(KLOOK)]
            cnt_regs = [
                nc.gpsimd.alloc_register(f"cnt_reg{i}") for i in range(2)
            ]
            for si, (b, j) in enumerate(seq):
                Tb = TB[b][j]
                tbb = tbb_of[(b, j)]
                rows_b = N_CORES * (bounds[b][1] - bounds[b][0])
                jsl = slice(j * 128, (j + 1) * 128)
                osl = slice(j * D_OUT, (j + 1) * D_OUT)
                msg2 = m2pool.tile([128, T2max * 128], bf16, tag="m2")
                # exact per-core valid-index count: the SWDGE ring bookkeeping
                # requires num_idxs_reg == number of descriptors generated
                # (a conservative padded count with -1 pads hangs the device)
                cnt_reg = cnt_regs[si % 2]
                nc.gpsimd.reg_load(cnt_reg, cnts_sb[0:1, si : si + 1])
                nc.gpsimd.dma_gather(
                    out_ap=msg2[:, : Tb * 128].rearrange(
                        "p (t e) -> p t e", e=128
                    ),
                    in_ap=p_full[b][0:rows_b, :],
                    idxs_ap=idx_sb[b][:, tbb * 8 : (tbb + Tb) * 8],
                    num_idxs=Tb * 128,
                    num_idxs_reg=cnt_reg,
                    elem_size=128,
                    single_packet=False,
                    queue_num=si % NQ,
                )
                oh = oh_ahead[si % KLOOK]
                if si + KLOOK < len(seq):
                    oh_ahead[si % KLOOK] = emit_oh2(si + KLOOK)
                pf = psA.tile([128, 128], f32, tag="agg")
                for ci in range(Tb):
                    nc.tensor.matmul(
                        out=pf[:],
                        lhsT=oh[:, ci * 128 : (ci + 1) * 128],
                        rhs=msg2[:, ci * 128 : (ci + 1) * 128],
                        start=(ci == 0),
                        stop=(ci == Tb - 1),
                    )
                if True:
                    if b == 0:
                        pd = psO.tile([128, D_OUT], f32, tag="p")
                        nc.tensor.matmul(
                            out=pd[:], lhsT=h_all[:, jsl], rhs=w2r_sb[:],
                            start=True, stop=False,
                        )
                        nc.tensor.matmul(
                            out=pd[:], lhsT=ones_sb[:], rhs=b2_sb[:],
                            start=False, stop=True,
                        )
                        pd_sb = spool.tile([128, D_OUT], f32, tag="pd_sb")
                        nc.scalar.copy(out=pd_sb[:], in_=pd[:])
                        nc.vector.scalar_tensor_tensor(
                            out=aggp[:, osl],
                            in0=pf[:, 0:D_OUT],
                            scalar=invc_sb[:, j : j + 1],
                            in1=pd_sb[:],
                            op0=mult,
                            op1=add,
                        )
                    elif b < NB - 1:
                        nc.vector.scalar_tensor_tensor(
                            out=aggp[:, osl],
                            in0=pf[:, 0:D_OUT],
                            scalar=invc_sb[:, j : j + 1],
                            in1=aggp[:, osl],
                            op0=mult,
                            op1=add,
                        )
                    else:
                        out_sb = spool.tile([128, D_OUT], f32, tag="out_sb")
                        nc.vector.scalar_tensor_tensor(
                            out=out_sb[:],
                            in0=pf[:, 0:D_OUT],
                            scalar=invc_sb[:, j : j + 1],
                            in1=aggp[:, osl],
                            op0=mult,
                            op1=add,
                        )
                        nc.sync.dma_start(out=out_d[jsl, :], in_=out_sb[:])

    nc.compile()
    return nc


def _prepare(x, edge_index, W1_l, b1, W1_r, W2_l, b2, W2_r):
    per_core, T1, TB = _preprocess(x, edge_index)
    nc = _build(T1, TB)
    shared = _shared_inputs(W1_l, b1, W1_r, W2_l, b2, W2_r)
    in_maps = [{**pc, **shared} for pc in per_core]
    return nc, in_maps


def kernel(
    x,
    edge_index,
    W1_l,
    b1,
    W1_r,
    W2_l,
    b2,
    W2_r,
):
    from concourse.bass_utils import run_bass_kernel_spmd

    nc, in_maps = _prepare(x, edge_index, W1_l, b1, W1_r, W2_l, b2, W2_r)
    res = run_bass_kernel_spmd(nc, in_maps, core_ids=list(range(N_CORES)))
    out = np.concatenate([r["out"] for r in res.results], axis=0)
    return out[:N].astype(np.float32)


if __name__ == "__main__":
    rng = np.random.default_rng(0)
    x = rng.standard_normal((N, D_IN), dtype=np.float32)
    ei = rng.integers(0, N, size=(2, E), dtype=np.int64)
    s = 1.0 / np.sqrt(D_IN)
    w1l = rng.uniform(-s, s, (D_IN, D_HID)).astype(np.float32)
    w1r = rng.uniform(-s, s, (D_IN, D_HID)).astype(np.float32)
    s2 = 1.0 / np.sqrt(D_HID)
    w2l = rng.uniform(-s2, s2, (D_HID, D_OUT)).astype(np.float32)
    w2r = rng.uniform(-s2, s2, (D_HID, D_OUT)).astype(np.float32)
    out = kernel(
        x=x,
        edge_index=ei,
        W1_l=w1l,
        b1=np.zeros(D_HID, np.float32),
        W1_r=w1r,
        W2_l=w2l,
        b2=np.zeros(D_OUT, np.float32),
        W2_r=w2r,
    )
    print(out.shape, out.dtype)
